# revision 1
# baseline (speedup 1.0000x reference)
"""AnyVariateAttention Trainium2 kernel (8 NeuronCores, SPMD).

Sharding: 16 (batch, head) pairs / 8 cores -> core c computes 2 adjacent heads
of batch c//4 (heads 2*(c%4), 2*(c%4)+1).

Host precomputes QKV projections + partial RoPE (cheap O(N*D^2) work) and the
final output projection; the device runs only the O(N^2) attention part:

- scores: S^T tiles [k=128, q=512] per 128-row k-chunk; q rows are pre-scaled
  by the Schraudolph constant A on the host, so PSUM holds y = A*s.
- exp: units of 2 chunks split between ACT (true exp: scale=1/A + per-class
  bias column, [128,1024] tiles) and DVE (round-to-int16 Schraudolph bit
  trick with the class bias folded into the scalar add, two [128,512]
  half-tiles whose shorter latency fits the smaller PSUM slot window).
- PV: q in PSUM partitions, out free dim = 33 (head-dim 32 + ones column for
  the softmax denominator), accumulated over 32 k-chunks.  Only the first
  matmul per q-tile carries start=True: it marks the whole PSUM bank
  pending-zero, and each slice's first touch then overwrites-as-zero
  (multiple start=True matmuls on one bank wipe earlier slices).
- out: unnormalized [pv|den] copied PSUM->SBUF and DMAd to DRAM; the host
  divides by the denominator and applies the output projection.
"""

import sys
import numpy as np

for _p in ("/opt/trn_rl_repo",):
    if _p not in sys.path:
        sys.path.insert(0, _p)

import ml_dtypes

BF16 = ml_dtypes.bfloat16

B, N, D, H, HD = 2, 4096, 256, 8, 32
SEQ = 512
SCALE = HD ** -0.5
NCORES = 8
SCHRAUD_A = 184.6650390625   # 128 * log2(e)
SCHRAUD_B0 = 16256.0         # exactly representable in bf16
SCHRAUD_ADJ = -7.4           # Schraudolph bias correction
ACT_SCALE = 1.0 / SCHRAUD_A
ACT_BIAS = -(SCHRAUD_B0 + SCHRAUD_ADJ) / SCHRAUD_A
ACT_FRAC = 0.554             # fraction of exp units on ACT engine

_NC_CACHE = {}


def _build_nc(stage=4):
    import concourse.bass as bass  # noqa: F401
    import concourse.tile as tile
    from concourse import bacc, mybir

    from concourse.alu_op_type import AluOpType
    bf = mybir.dt.bfloat16
    f32 = mybir.dt.float32
    i16 = mybir.dt.int16
    EXP = mybir.ActivationFunctionType.Exp

    nc = bacc.Bacc("TRN2", target_bir_lowering=False, debug=False,
                   num_devices=NCORES)

    q_d = nc.declare_dram_parameter("q", [64, N], bf, isOutput=False)
    k_d = nc.declare_dram_parameter("k", [64, N], bf, isOutput=False)
    v_d = nc.declare_dram_parameter("v", [128, 32 * 2 * 33], bf, isOutput=False)
    # bias cols 0-3: DVE (A*b + B0 + adj), cols 4-7: ACT (b); col = 2h+cls
    bias_d = nc.declare_dram_parameter("biases", [128, 8], f32, isOutput=False)
    out_d = nc.declare_dram_parameter("out", [128, 8 * 264], f32, isOutput=True)

    NT = N // 512        # 8 q-tiles of 512
    NCP = 16             # 16 chunk-pairs of 2x128 k rows per (h, t)

    # global tile order: for t, for h, for p
    tiles = [(t, h, p) for t in range(NT) for h in range(2) for p in range(NCP)]
    n_tiles = len(tiles)

    # Bresenham route assignment: 0 = ACT [128,1024] (spa),
    # 2 = DVE 2x[128,512] (spd)
    routes = []
    acc = 0.0
    for _ in range(n_tiles):
        acc += ACT_FRAC
        if acc >= 1.0:
            acc -= 1.0
            routes.append(0)
        else:
            routes.append(2)

    with tile.TileContext(nc) as tc:
        from contextlib import ExitStack

        with ExitStack() as ctx:
            const = ctx.enter_context(tc.tile_pool(name="const", bufs=1))

            q_sb = const.tile([64, N], bf, tag="q_sb")
            k_sb = const.tile([64, N], bf, tag="k_sb")
            v_sb = const.tile([128, 32 * 2 * 33], bf, tag="v_sb")
            bias_sb = const.tile([128, 8], f32, tag="bias_sb")

            # split input DMAs so the first tiles' operands land early
            nc.sync.dma_start(k_sb[:, 0:1024], k_d[:, 0:1024])
            nc.sync.dma_start(q_sb[:, 0:512], q_d[:, 0:512])
            nc.sync.dma_start(bias_sb[:], bias_d[:])
            nc.sync.dma_start(v_sb[:, 0:528], v_d[:, 0:528])
            nc.sync.dma_start(k_sb[:, 1024:N], k_d[:, 1024:N])
            nc.sync.dma_start(q_sb[:, 512:N], q_d[:, 512:N])
            nc.sync.dma_start(v_sb[:, 528:2112], v_d[:, 528:2112])

            # ACT-routed tiles: [128,1024] (2 banks x 2 slots); DVE-routed
            # tiles: two [128,512] halves (1 bank x 2 slots) — the shorter
            # DVE exp latency fits the slot-recycle window.
            spa = ctx.enter_context(
                tc.tile_pool(name="spa", bufs=2, space="PSUM"))
            spd = ctx.enter_context(
                tc.tile_pool(name="spd", bufs=3, space="PSUM"))
            pvp = ctx.enter_context(
                tc.tile_pool(name="pvp", bufs=1, space="PSUM"))
            ptp = ctx.enter_context(tc.tile_pool(name="ptp", bufs=8))
            osp = ctx.enter_context(tc.tile_pool(name="osp", bufs=2))

            sp_tiles = {}   # step -> sp tile
            pt_tiles = {}   # step -> pt AP (bf16 view)
            pv_tiles = {}   # t -> pv psum tile

            def emit_scores(s):
                t, h, p = tiles[s]
                base = 32 * h
                if routes[s] < 2:
                    sp = spa.tile([128, 1024], f32, tag="sp", name=f"sp{s}")
                    sp_tiles[s] = (sp,)
                    for j in range(2):
                        c = 2 * p + j
                        nc.tensor.matmul(
                            sp[:, j * 512:(j + 1) * 512],
                            lhsT=k_sb[base:base + 32, c * 128:(c + 1) * 128],
                            rhs=q_sb[base:base + 32, t * 512:(t + 1) * 512],
                            start=True, stop=True)
                else:
                    halves = []
                    for j in range(2):
                        c = 2 * p + j
                        sp = spd.tile([128, 512], f32, tag="spd",
                                      name=f"sp{s}_{j}")
                        nc.tensor.matmul(
                            sp[:],
                            lhsT=k_sb[base:base + 32, c * 128:(c + 1) * 128],
                            rhs=q_sb[base:base + 32, t * 512:(t + 1) * 512],
                            start=True, stop=True)
                        halves.append(sp)
                    sp_tiles[s] = tuple(halves)

            def emit_exp(s):
                t, h, p = tiles[s]
                # class: same-variate iff k-variate (p//2) == q-variate (t)
                col = 2 * h + (0 if (p // 2) == t else 1)
                sps = sp_tiles.pop(s)
                if routes[s] == 0:
                    sp, = sps
                    pt = ptp.tile([128, 1024], bf, tag="pt", name=f"pt{s}")
                    nc.scalar.activation(
                        pt[:], sp[:], EXP, bias=bias_sb[:, 4 + col:5 + col],
                        scale=ACT_SCALE)
                    pt_tiles[s] = (pt[:],)
                elif routes[s] == 1:
                    sp, = sps
                    pt = ptp.tile([128, 1024], i16, tag="pt", name=f"pte{s}")
                    nc.vector.tensor_scalar(
                        pt[:], sp[:], 1.0, bias_sb[:, col:col + 1],
                        AluOpType.mult, AluOpType.add)
                    pt_tiles[s] = (pt[:].bitcast(bf),)
                else:
                    outs = []
                    for j, sp in enumerate(sps):
                        pt = ptp.tile([128, 512], i16, tag="pti",
                                      name=f"pti{s}_{j}")
                        nc.vector.tensor_scalar(
                            pt[:], sp[:], 1.0, bias_sb[:, col:col + 1],
                            AluOpType.mult, AluOpType.add)
                        outs.append(pt[:].bitcast(bf))
                    pt_tiles[s] = tuple(outs)

            def emit_pv(s):
                t, h, p = tiles[s]
                if h == 0 and p == 0:
                    pv_tiles[t] = pvp.tile([128, 264], f32, tag="pv",
                                           name=f"pv{t}")
                pv = pv_tiles[t]
                pts = pt_tiles.pop(s)
                for j in range(2):
                    c = 2 * p + j
                    if len(pts) == 1:
                        src = pts[0]
                        off = j * 512
                    else:
                        src = pts[j]
                        off = 0
                    for qc in range(4):
                        # One start=True per t: it marks the whole PSUM bank
                        # pending-zero (ZERO_REGION_SIZE=2KB covers all 8
                        # slices); every other slice's first touch then
                        # overwrites-as-zero. Multiple start=True matmuls on
                        # the same bank would wipe earlier slices' data.
                        first = (h == 0 and c == 0 and qc == 0)
                        nc.tensor.matmul(
                            pv[:, (h * 4 + qc) * 33:(h * 4 + qc + 1) * 33],
                            lhsT=src[:, off + qc * 128:off + (qc + 1) * 128],
                            rhs=v_sb[:, (c * 2 + h) * 33:(c * 2 + h + 1) * 33],
                            start=first, stop=(c == 31),
                            skip_group_check=True)

            def emit_out(t):
                pv = pv_tiles.pop(t)
                ot = osp.tile([128, 264], f32, tag="ot", name=f"ot{t}")
                nc.scalar.copy(ot[:], pv[:])
                nc.sync.dma_start(out_d[:, t * 264:(t + 1) * 264], ot[:])

            # software pipeline: scores(s), PV lagging 4 steps (deep lag
            # absorbs exp-latency jitter; pt pool buffers it), exp(s-1);
            # the out-copy for a finished t trails one more step.
            for s in range(n_tiles + 6):
                if s < n_tiles:
                    emit_scores(s)
                if 0 <= s - 4 < n_tiles:
                    emit_pv(s - 4)
                if 0 <= s - 1 < n_tiles:
                    emit_exp(s - 1)
                so = s - 5
                if 0 <= so < n_tiles:
                    t, h, p = tiles[so]
                    if h == 1 and p == NCP - 1:
                        emit_out(t)

    nc.compile()
    return nc


def _rope(x, positions):
    # x: [..., N, hd]; partial RoPE (rope_percent=0.5)
    half = HD // 2
    ra = half // 2
    frac = 2.0 * np.arange(ra, dtype=np.float32) / HD
    ts = (10000.0 ** frac).astype(np.float32)
    sinu = positions[:, None] / ts[None, :]
    sin = np.sin(sinu).astype(np.float32)
    cos = np.cos(sinu).astype(np.float32)
    f, s = x[..., :half], x[..., half:]
    fr, fp = f[..., :ra], f[..., ra:]
    sr, sp = s[..., :ra], s[..., ra:]
    return np.concatenate(
        [fr * cos - sr * sin, fp, sr * cos + fr * sin, sp], axis=-1)


def kernel(**inputs):
    hs = np.asarray(inputs["hidden_states"], dtype=np.float32)
    qw = np.asarray(inputs["q_w"], dtype=np.float32)
    kw = np.asarray(inputs["k_w"], dtype=np.float32)
    vw = np.asarray(inputs["v_w"], dtype=np.float32)
    ow = np.asarray(inputs["o_w"], dtype=np.float32)
    obb = np.asarray(inputs["o_b"], dtype=np.float32)
    qb_ = np.asarray(inputs["q_b"], dtype=np.float32)
    kb_ = np.asarray(inputs["k_b"], dtype=np.float32)
    vb_ = np.asarray(inputs["v_b"], dtype=np.float32)
    ab = np.asarray(inputs["attention_biases"], dtype=np.float32)
    seq = int(np.asarray(inputs["sequence_length"]))
    assert seq == SEQ, f"kernel compiled for sequence_length={SEQ}, got {seq}"
    assert hs.shape == (B, N, D)

    if ("nc", 4) not in _NC_CACHE:
        _NC_CACHE[("nc", 4)] = _build_nc(4)
    nc = _NC_CACHE[("nc", 4)]

    # host-side projections + rope (f32)
    pos = np.arange(N, dtype=np.float32)
    q = (hs @ qw.T + qb_) * SCALE    # [B, N, D]
    k = hs @ kw.T + kb_
    v = hs @ vw.T + vb_
    q = q.reshape(B, N, H, HD).transpose(0, 2, 1, 3)  # [B, H, N, hd]
    k = k.reshape(B, N, H, HD).transpose(0, 2, 1, 3)
    v = v.reshape(B, N, H, HD).transpose(0, 2, 1, 3)
    q = _rope(q, pos)
    k = _rope(k, pos)

    in_maps = []
    for c in range(NCORES):
        b = c // 4
        h0 = 2 * (c % 4)
        q_t = np.empty((64, N), dtype=np.float32)
        k_t = np.empty((64, N), dtype=np.float32)
        v_t = np.empty((128, 32, 2, 33), dtype=np.float32)
        bias_t = np.empty((128, 8), dtype=np.float32)
        for j in range(2):
            h = h0 + j
            q_t[32 * j:32 * j + 32, :] = (SCHRAUD_A * q[b, h]).T
            k_t[32 * j:32 * j + 32, :] = k[b, h].T
            v_t[:, :, j, :32] = v[b, h].reshape(32, 128, 32).transpose(1, 0, 2)
            v_t[:, :, j, 32] = 1.0
            for cls in range(2):  # 0 = same, 1 = diff
                bias_t[:, 2 * j + cls] = (SCHRAUD_A * ab[h, cls]
                                          + SCHRAUD_B0 + SCHRAUD_ADJ)
                bias_t[:, 4 + 2 * j + cls] = ab[h, cls]
        in_maps.append({
            "q": q_t.astype(BF16),
            "k": k_t.astype(BF16),
            "v": np.ascontiguousarray(v_t.reshape(128, 32 * 2 * 33)).astype(BF16),
            "biases": bias_t,
        })

    global _LAST_IN_MAPS, _LAST_RESULTS
    _LAST_IN_MAPS = in_maps
    from concourse.bass_utils import run_bass_kernel_spmd
    res = run_bass_kernel_spmd(nc, in_maps, core_ids=list(range(NCORES)))
    _LAST_RESULTS = res.results

    attn = np.empty((B, H, N, HD), dtype=np.float32)
    for c in range(NCORES):
        b = c // 4
        h0 = 2 * (c % 4)
        o = res.results[c]["out"].reshape(128, 8, 2, 4, 33)
        for j in range(2):
            # q = 512*t + 128*qc + row
            pv = o[:, :, j, :, :32]    # [row, t, qc, 32]
            den = o[:, :, j, :, 32]    # [row, t, qc]
            x = pv / den[..., None]
            attn[b, h0 + j] = x.transpose(1, 2, 0, 3).reshape(N, HD)

    ctx = attn.transpose(0, 2, 1, 3).reshape(B, N, D)
    return ctx @ ow.T + obb[None, None, :]

